# revision 1
# baseline (speedup 1.0000x reference)
"""Trainium2 Bass kernel for nn_CustomLayerMKM: y = x @ (sum_k kron(Bk, Ak)).T + bias.

Exploits the Kronecker structure instead of materializing the dense 4096x4096
weight: kron(Bk,Ak) = kron(Bk,I) @ kron(I,Ak), so each factor costs two cheap
matmul stages (~9x fewer FLOPs than dense).

Sharding: data-parallel over B across 8 cores (512 rows each); the small
Kronecker factors are replicated. No collectives.

Host prep (cheap, O(B*I) element moves): x is pre-transposed + cast to bf16,
laid out quarter-contiguous per core; the 128x128 "pattern" matrices (permuted
copies of the factor weights) are built in numpy and passed as inputs; bias is
added on the host after the gather.

Per-core device pipeline, software-pipelined over 4 b-quarters of 128 rows
(U is double-buffered so quarter q+1's stage 1 overlaps quarter q's
corner-turn + stage 2):
  stage 1: per 128-wide i-block t: U_k = xT_block.T @ patA_k   (PE, N=128)
           U_k free index fidx = u*128 + w*f1 + t*G + g  (u = q mod 32)
  corner-turn: V_k = U_k.T via DMA-xbar transpose (bf16, 1 DMA per (k,q),
           alternating between the two HWDGE queues)
  stage 2: per output group u: one PSUM tile accumulates all 3 factors
           (lhsT = V_k[:, u, :], rhs = patB_k), then one strided eviction
           writes y columns o = c*32 + u.
"""

from contextlib import ExitStack

import numpy as np

P = 128
B_FULL, I_DIM, O_DIM = 4096, 4096, 4096
N_CORES = 8
B_SHARD = B_FULL // N_CORES          # 512 rows per core
NQ = 4                               # b-shard processed in 4 quarters of 128
FACTOR_DIMS = [(64, 64), (128, 32), (32, 128)]   # (m, f1) per factor
N_FAC = 3
TB = I_DIM // P                      # 32 i-blocks
UG = 32                              # output groups u = q mod 32
MM_DTYPE = "bfloat16"


def build_nc(debug_dump=False):
    import concourse.bass as bass
    import concourse.mybir as mybir
    import concourse.tile as tile
    from concourse import bacc

    MM_DT = getattr(mybir.dt, MM_DTYPE)
    F32 = mybir.dt.float32
    ts = bass.ts

    nc = bacc.Bacc("TRN2", target_bir_lowering=False, debug=False,
                   num_devices=N_CORES)

    # xT laid out quarter-major: [q, 4096, 128]
    xT_ext = nc.dram_tensor("xT", [NQ, I_DIM, P], MM_DT,
                            kind="ExternalInput").ap()
    pat_ext = {}
    for k in range(N_FAC):
        for nm in ("patA", "patB"):
            pat_ext[f"{nm}{k}"] = nc.dram_tensor(
                f"{nm}{k}", [P, P], MM_DT, kind="ExternalInput").ap()
    y_ext = nc.dram_tensor("y", [B_SHARD, O_DIM], F32,
                           kind="ExternalOutput").ap()

    with tile.TileContext(nc) as tc, ExitStack() as ctx:
        const = ctx.enter_context(tc.tile_pool(name="const", bufs=1))
        ps = ctx.enter_context(tc.tile_pool(name="ps", bufs=8, space="PSUM"))
        xtp = ctx.enter_context(tc.tile_pool(name="xtp", bufs=2))
        upool = ctx.enter_context(tc.tile_pool(name="upool", bufs=2))
        vpool = ctx.enter_context(tc.tile_pool(name="vpool", bufs=2))
        ypool = ctx.enter_context(tc.tile_pool(name="ypool", bufs=2))

        patA, patB = [], []
        for k in range(N_FAC):
            pa = const.tile([P, P], MM_DT, tag=f"patA{k}")
            nc.sync.dma_start(pa[:], pat_ext[f"patA{k}"][:])
            pb = const.tile([P, P], MM_DT, tag=f"patB{k}")
            nc.sync.dma_start(pb[:], pat_ext[f"patB{k}"][:])
            patA.append(pa)
            patB.append(pb)

        n_ev = [0]

        def evict(dst, src):
            if n_ev[0] % 2 == 0:
                nc.vector.tensor_copy(dst, src)
            else:
                nc.scalar.copy(dst, src)
            n_ev[0] += 1

        n_tp = [0]

        def dma_transpose(dst, src):
            nc.sync.dma_start_transpose(dst, src)
            n_tp[0] += 1

        for q in range(NQ):
            # ---- load this quarter's xT (contiguous 1MB) ----
            xT_sb = xtp.tile([P, TB, P], MM_DT, tag="xT", name=f"xT{q}")
            nc.sync.dma_start(
                xT_sb[:],
                xT_ext[q].rearrange("(t p) b -> p t b", p=P, t=TB))

            # ---- stage 1 (bf16 PSUM -> 2x evictions) ----
            U = [upool.tile([P, I_DIM], MM_DT, tag=f"U{k}", name=f"U{q}_{k}")
                 for k in range(N_FAC)]
            for T in range(TB // 4):
                s1 = [ps.tile([P, 512], F32, tag="ps",
                              name=f"s1_{q}_{T}_{kk}")
                      for kk in range(N_FAC)]
                for tl in range(4):
                    lhsT = xT_sb[:, 4 * T + tl, :]
                    for k in range(N_FAC):
                        nc.tensor.matmul(s1[k][:, ts(tl, P)], lhsT,
                                         patA[k][:], start=True, stop=True)
                # src col c = u*4 + w*G + g within each tl-region
                u0 = U[0].rearrange("p (u w t2 tl g) -> p w u tl g t2",
                                    u=32, w=2, t2=8, tl=4, g=2)
                s0 = s1[0].rearrange("p (tl u w g) -> p w u tl g",
                                     tl=4, u=32, w=2, g=2)
                for w in range(2):
                    evict(u0[:, w, :, :, :, T], s0[:, w])
                u1 = U[1].rearrange("p (u w t2 tl) -> p w u tl t2",
                                    u=32, w=4, t2=8, tl=4)
                s_1 = s1[1].rearrange("p (tl u w) -> p w u tl",
                                      tl=4, u=32, w=4)
                evict(u1[:, :, :, :, T], s_1[:, :])
                u2 = U[2].rearrange("p (u t2 tl g) -> p u tl g t2",
                                    u=32, t2=8, tl=4, g=4)
                s_2 = s1[2].rearrange("p (tl u g) -> p u tl g",
                                      tl=4, u=32, g=4)
                evict(u2[:, :, :, :, T], s_2[:, :])

            # ---- corner-turn via DMA-xbar transpose ----
            V = [vpool.tile([P, TB, P], MM_DT, tag=f"V{k}", name=f"V{q}_{k}")
                 for k in range(N_FAC)]
            for k in range(N_FAC):
                dma_transpose(V[k][:], U[k][:])

            # ---- stage 2 ----
            y_q = ypool.tile([P, O_DIM], F32, tag="yq", name=f"yq{q}")
            for Ug4 in range(UG // 4):
                y_ps = ps.tile([P, 512], F32, tag="ps", name=f"yps{q}_{Ug4}")
                for ul in range(4):
                    u = Ug4 * 4 + ul
                    for k in range(N_FAC):
                        nc.tensor.matmul(
                            y_ps[:, ts(ul, P)],
                            V[k][:, u, :],
                            patB[k][:],
                            start=(k == 0), stop=(k == N_FAC - 1))
                dst = y_q.rearrange("p (c u) -> p u c", c=P,
                                    u=UG)[:, Ug4 * 4:Ug4 * 4 + 4, :]
                evict(dst, y_ps.rearrange("p (ul c) -> p ul c", ul=4, c=P))

            nc.sync.dma_start(y_ext[q * P:(q + 1) * P, :], y_q[:])

    nc.compile()
    return nc


_NC_CACHE = {}


def prep_inputs(inputs):
    """Host preprocessing: per-core bf16 quarter-major xT + pattern matrices."""
    import ml_dtypes

    bf16 = ml_dtypes.bfloat16
    x = np.asarray(inputs["input_BI"], dtype=np.float32)
    As = [np.asarray(inputs[n], dtype=np.float32) for n in ("w0a", "w1a", "w2a")]
    Bs = [np.asarray(inputs[n], dtype=np.float32) for n in ("w0b", "w1b", "w2b")]

    common = {}
    for k, ((m, f1), A, Bk) in enumerate(zip(FACTOR_DIMS, As, Bs)):
        G, H = P // m, P // f1
        pa = np.zeros((P, P), np.float32)
        q_uw = np.arange(32)[:, None] + 32 * np.arange(H)[None, :]
        cols = (np.arange(32)[:, None] * H * G + np.arange(H)[None, :] * G)
        for g in range(G):
            pa[g * m:(g + 1) * m, (cols + g).ravel()] = A[q_uw.ravel(), :].T
        pb = np.zeros((P, P), np.float32)
        f2 = Bk.shape[0]
        for wp in range(H):
            pb[wp * f1:(wp + 1) * f1, np.arange(f2) * H + wp] = Bk.T
        common[f"patA{k}"] = np.ascontiguousarray(pa.astype(bf16))
        common[f"patB{k}"] = np.ascontiguousarray(pb.astype(bf16))

    in_maps = []
    for c in range(N_CORES):
        im = dict(common)
        xs = x[c * B_SHARD:(c + 1) * B_SHARD].T.astype(bf16)   # (4096, 512)
        im["xT"] = np.ascontiguousarray(
            xs.reshape(I_DIM, NQ, P).transpose(1, 0, 2))       # (4, 4096, 128)
        in_maps.append(im)
    return in_maps


def kernel(**inputs):
    """Full-input entry point: shards over B, runs 8-core SPMD, gathers."""
    from concourse.bass_utils import run_bass_kernel_spmd

    in_maps = prep_inputs(inputs)
    if "nc" not in _NC_CACHE:
        _NC_CACHE["nc"] = build_nc()
    res = run_bass_kernel_spmd(_NC_CACHE["nc"], in_maps,
                               core_ids=list(range(N_CORES)))
    y = np.concatenate([r["y"] for r in res.results], axis=0)
    return y + np.asarray(inputs["bias_O"], dtype=np.float32)[None, :]



# revision 16
# speedup vs baseline: 1.3848x; 1.3848x over previous
"""Trainium2 Bass kernel for nn_CustomLayerMKM: y = x @ (sum_k kron(Bk, Ak)).T + bias.

Exploits the Kronecker structure: per factor, y_b = Bk @ X_b @ Ak^T via two
matmul stages with a DMA-xbar corner-turn between them (~9x fewer FLOPs than
dense).

Sharding: data-parallel over B across 8 cores (512 rows each, processed as 4
b-quarters of 128); Kronecker factors replicated. No collectives.

v2 layout (vs v1): stage-2 uses pattern-stationary matmuls whose output
partition is s = o//32 and PSUM tile r = o%32, letting all 3 factors
accumulate into ONE psum bank (single start flag; has_written semantics give
overwrite-then-accumulate). This cuts stage-2 evictions 3x, removes per-u
weight reloads, and the output is written bf16 (y^T staging layout, host
reassembles + bias). Software pipeline interleaves stage-2 of quarter q-1
after stage-1 of quarter q so the PE never waits on the corner-turn DMAs,
which alternate between the two HWDGE queues (sync + scalar).

Index map per factor k with wa:(p,q), wb:(f2,f1), j=i_full//q_k, l=i_full%q_k:
  stage-1 psum col (per i-block t) c = r*4 + e, e encodes (ko//32, j%G)
  U free = r*128 + t*4 + e;  per-128-block DMA transpose -> V[p', r, b]
  p' = t*4+e = {2j+kob | 4j+kob | j} for k={0,1,2}, ko = kob*32 + r
  stage-2: yps[s, b] += patB2_k.T @ V_k[:, r, :], s = o//32, o = 32s + r
"""

from contextlib import ExitStack

import numpy as np

P = 128
B_FULL, I_DIM, O_DIM = 4096, 4096, 4096
N_CORES = 8
B_SHARD = B_FULL // N_CORES          # 512 rows per core
NQ = 4                               # b-shard processed in 4 quarters of 128
N_FAC = 3
TB = I_DIM // P                      # 32 i-blocks
MM_DTYPE = "bfloat16"


def build_nc(tp_mode="sync", s2_mode="fused"):
    """tp_mode: 'split' (half-tiles, alternate sync/scalar HWDGE),
    'alt' (full tiles, alternate engines), 'sync' (full tiles, sync only).
    s2_mode: 'fused' (one psum group for all 3 factors, single start) or
    'groups' (per-rl accumulation groups, baseline-proven semantics)."""
    import concourse.bass as bass
    import concourse.mybir as mybir
    import concourse.tile as tile
    from concourse import bacc

    MM_DT = getattr(mybir.dt, MM_DTYPE)
    F32 = mybir.dt.float32
    ts = bass.ts

    nc = bacc.Bacc("TRN2", target_bir_lowering=False, debug=False,
                   num_devices=N_CORES)

    # xT laid out [q, pp, t, b]: the per-quarter load is a linear 1MB DMA
    xT_ext = nc.dram_tensor("xT", [NQ, P, TB * P], MM_DT,
                            kind="ExternalInput").ap()
    pat_ext = {}
    for k in range(N_FAC):
        for nm in ("patA", "patB"):
            pat_ext[f"{nm}{k}"] = nc.dram_tensor(
                f"{nm}{k}", [P, P], MM_DT, kind="ExternalInput").ap()
    # y staging: [q, s, r*128 + b] bf16; host: y[q*128+b, 32s+r]
    y_ext = nc.dram_tensor("y", [NQ, P, TB * P], MM_DT,
                           kind="ExternalOutput").ap()

    with tile.TileContext(nc) as tc, ExitStack() as ctx:
        const = ctx.enter_context(tc.tile_pool(name="const", bufs=1))
        ps = ctx.enter_context(tc.tile_pool(name="ps", bufs=8, space="PSUM"))
        xtp = ctx.enter_context(tc.tile_pool(name="xtp", bufs=4))
        upool = ctx.enter_context(tc.tile_pool(name="upool", bufs=2))
        vpool = ctx.enter_context(tc.tile_pool(name="vpool", bufs=2))
        ypool = ctx.enter_context(tc.tile_pool(name="ypool", bufs=2))

        patA, patB = [], []
        for k in range(N_FAC):
            pa = const.tile([P, P], MM_DT, tag=f"patA{k}")
            nc.sync.dma_start(pa[:], pat_ext[f"patA{k}"][:])
            pb = const.tile([P, P], MM_DT, tag=f"patB{k}")
            nc.sync.dma_start(pb[:], pat_ext[f"patB{k}"][:])
            patA.append(pa)
            patB.append(pb)

        n_ev = [0]

        def evict(dst, src):
            if n_ev[0] % 2 == 0:
                nc.vector.tensor_copy(dst, src)
            else:
                nc.scalar.copy(dst, src)
            n_ev[0] += 1

        n_tp = [0]

        def dma_transpose(dst, src):
            eng = nc.sync if (tp_mode == "sync" or n_tp[0] % 2 == 0) else nc.scalar
            eng.dma_start_transpose(dst, src)
            n_tp[0] += 1

        def load_x(q):
            # sync ring, same as transposes: SBUF-writing DMAs must never
            # run concurrently with the xbar-transpose S2M path (HW hazard);
            # same-ring FIFO serializes them. All 4 quarters are prefetched
            # up front so transposes have the ring to themselves afterward.
            xT_sb = xtp.tile([P, TB, P], MM_DT, tag="xT", name=f"xT{q}")
            nc.sync.dma_start(xT_sb[:], xT_ext[q])
            return xT_sb

        def stage1(q, xT_sb):
            # one U slab for all 3 factors -> one 3MB transpose per quarter
            U = upool.tile([P, N_FAC * TB * P], MM_DT, tag="U", name=f"U{q}")
            for T in range(TB // 4):
                s1 = [ps.tile([P, 512], F32, tag="ps",
                              name=f"s1_{q}_{T}_{kk}")
                      for kk in range(N_FAC)]
                for tl in range(4):
                    lhsT = xT_sb[:, 4 * T + tl, :]
                    for k in range(N_FAC):
                        nc.tensor.matmul(s1[k][:, ts(tl, P)], lhsT,
                                         patA[k][:], start=True, stop=True)
                # evict: U[b, k*4096 + r*128 + t*4 + e] = s1_k[b, tl*128+r*4+e]
                # iterate (r, tl, e): dst runs are 16 contiguous elems (32B)
                for k in range(N_FAC):
                    src = s1[k].rearrange("p (tl r e) -> p r tl e",
                                          tl=4, r=32, e=4)
                    dst = U.rearrange("p (k r t e) -> p k r t e",
                                      k=N_FAC, r=32, t=TB,
                                      e=4)[:, k, :, 4 * T:4 * T + 4]
                    evict(dst, src)
            V = vpool.tile([P, N_FAC * TB, P], MM_DT, tag="V", name=f"V{q}")
            if tp_mode == "split":
                dma_transpose(V[:, :N_FAC * TB // 2], U[:, :N_FAC * TB * P // 2])
                dma_transpose(V[:, N_FAC * TB // 2:], U[:, N_FAC * TB * P // 2:])
            else:
                dma_transpose(V[:], U[:])
            return V

        def stage2(q, V):
            y_q = ypool.tile([P, TB * P], MM_DT, tag="yq", name=f"yq{q}")
            for R in range(8):
                yps = ps.tile([P, 512], F32, tag="ps", name=f"yps{q}_{R}")
                if s2_mode == "fused":
                    for k in range(N_FAC):
                        for rl in range(4):
                            r = R * 4 + rl
                            nc.tensor.matmul(
                                yps[:, ts(rl, P)],
                                patB[k][:],
                                V[:, k * TB + r, :],
                                start=(k == 0 and rl == 0),
                                stop=(k == N_FAC - 1 and rl == 3),
                                skip_group_check=True)
                else:
                    for rl in range(4):
                        r = R * 4 + rl
                        for k in range(N_FAC):
                            nc.tensor.matmul(
                                yps[:, ts(rl, P)],
                                patB[k][:],
                                V[:, k * TB + r, :],
                                start=(k == 0),
                                stop=(k == N_FAC - 1))
                evict(y_q[:, ts(R, 512)], yps[:])
            nc.scalar.dma_start(y_ext[q], y_q[:])

        # software pipeline: s1(0), [s1(q+1) || s2(q)], s2(3)
        xT = [load_x(q) for q in range(NQ)]
        V_prev = None
        for q in range(NQ):
            V_cur = stage1(q, xT[q])
            if V_prev is not None:
                stage2(q - 1, V_prev)
            V_prev = V_cur
        stage2(NQ - 1, V_prev)

    nc.compile()
    return nc


_NC_CACHE = {}


def prep_inputs(inputs):
    """Host preprocessing: per-core bf16 quarter-major xT + pattern matrices."""
    import ml_dtypes

    bf16 = ml_dtypes.bfloat16
    x = np.asarray(inputs["input_BI"], dtype=np.float32)
    As = [np.asarray(inputs[n], dtype=np.float32) for n in ("w0a", "w1a", "w2a")]
    Bs = [np.asarray(inputs[n], dtype=np.float32) for n in ("w0b", "w1b", "w2b")]

    common = {}
    # patA_k[pp, r*4+e]; see module docstring for the index map
    pa0 = np.zeros((2, 64, 32, 2, 2), np.float32)      # [g, l, r, g', kob]
    w0 = As[0].reshape(2, 32, 64).transpose(2, 1, 0)   # [l, r, kob]
    for g in range(2):
        pa0[g, :, :, g, :] = w0
    common["patA0"] = pa0.reshape(P, P)
    common["patA1"] = As[1].reshape(4, 32, P).transpose(2, 1, 0).reshape(P, P)
    pa2 = np.zeros((4, 32, 32, 4), np.float32)         # [g, l, r, g']
    for g in range(4):
        pa2[g, :, :, g] = As[2].T                       # [l, r] (ko = r)
    common["patA2"] = pa2.reshape(P, P)

    # patB2_k[p', s]
    pb0 = np.zeros((64, 2, 64, 2), np.float32)         # [j, kob, i, kob']
    for kob in range(2):
        pb0[:, kob, :, kob] = Bs[0].T
    common["patB0"] = pb0.reshape(P, P)
    pb1 = np.zeros((32, 4, 32, 4), np.float32)
    for kob in range(4):
        pb1[:, kob, :, kob] = Bs[1].T
    common["patB1"] = pb1.reshape(P, P)
    common["patB2"] = np.ascontiguousarray(Bs[2].T)

    for k in list(common):
        common[k] = np.ascontiguousarray(common[k].astype(bf16))

    in_maps = []
    for c in range(N_CORES):
        im = dict(common)
        xs = x[c * B_SHARD:(c + 1) * B_SHARD].T.astype(bf16)   # (4096, 512)
        # [q, pp, t, b]: per-quarter DMA is a linear [128, 4096] copy
        im["xT"] = np.ascontiguousarray(
            xs.reshape(TB, P, NQ, P).transpose(2, 1, 0, 3)
            .reshape(NQ, P, TB * P))
        in_maps.append(im)
    return in_maps


def finish_output(res_list, bias):
    """Reassemble [q,s,r,b] bf16 staging -> [B, O] f32 + bias."""
    outs = []
    for r in res_list:
        ystage = np.asarray(r["y"]).reshape(NQ, P, TB, P)
        y_core = ystage.transpose(0, 3, 1, 2).reshape(B_SHARD, O_DIM)
        outs.append(y_core.astype(np.float32))
    y = np.concatenate(outs, axis=0)
    return y + bias[None, :]


def kernel(**inputs):
    """Full-input entry point: shards over B, runs 8-core SPMD, gathers."""
    from concourse.bass_utils import run_bass_kernel_spmd

    in_maps = prep_inputs(inputs)
    if "nc" not in _NC_CACHE:
        _NC_CACHE["nc"] = build_nc()
    res = run_bass_kernel_spmd(_NC_CACHE["nc"], in_maps,
                               core_ids=list(range(N_CORES)))
    bias = np.asarray(inputs["bias_O"], dtype=np.float32)
    return finish_output(res.results, bias)


# revision 20
# speedup vs baseline: 1.4054x; 1.0148x over previous
"""Trainium2 Bass kernel for nn_CustomLayerMKM: y = x @ (sum_k kron(Bk, Ak)).T + bias.

Exploits the Kronecker structure: per factor, y_b = Bk @ X_b @ Ak^T via two
matmul stages with a DMA-xbar corner-turn between them (~9x fewer FLOPs than
dense).

Sharding: data-parallel over B across 8 cores (512 rows each, processed as 4
b-quarters of 128); Kronecker factors replicated. No collectives.

v2 layout (vs v1): stage-2 uses pattern-stationary matmuls whose output
partition is s = o//32 and PSUM tile r = o%32, letting all 3 factors
accumulate into ONE psum bank (single start flag; has_written semantics give
overwrite-then-accumulate). This cuts stage-2 evictions 3x, removes per-u
weight reloads, and the output is written bf16 (y^T staging layout, host
reassembles + bias). Software pipeline interleaves stage-2 of quarter q-1
after stage-1 of quarter q so the PE never waits on the corner-turn DMAs,
which alternate between the two HWDGE queues (sync + scalar).

Index map per factor k with wa:(p,q), wb:(f2,f1), j=i_full//q_k, l=i_full%q_k:
  stage-1 psum col (per i-block t) c = r*4 + e, e encodes (ko//32, j%G)
  U free = r*128 + t*4 + e;  per-128-block DMA transpose -> V[p', r, b]
  p' = t*4+e = {2j+kob | 4j+kob | j} for k={0,1,2}, ko = kob*32 + r
  stage-2: yps[s, b] += patB2_k.T @ V_k[:, r, :], s = o//32, o = 32s + r
"""

from contextlib import ExitStack

import numpy as np

P = 128
B_FULL, I_DIM, O_DIM = 4096, 4096, 4096
N_CORES = 8
B_SHARD = B_FULL // N_CORES          # 512 rows per core
NQ = 4                               # b-shard processed in 4 quarters of 128
N_FAC = 3
TB = I_DIM // P                      # 32 i-blocks
MM_DTYPE = "bfloat16"


def build_nc(tp_mode="sync", s2_mode="fused"):
    """tp_mode: 'split' (half-tiles, alternate sync/scalar HWDGE),
    'alt' (full tiles, alternate engines), 'sync' (full tiles, sync only).
    s2_mode: 'fused' (one psum group for all 3 factors, single start) or
    'groups' (per-rl accumulation groups, baseline-proven semantics)."""
    import concourse.bass as bass
    import concourse.mybir as mybir
    import concourse.tile as tile
    from concourse import bacc

    MM_DT = getattr(mybir.dt, MM_DTYPE)
    F32 = mybir.dt.float32
    ts = bass.ts

    nc = bacc.Bacc("TRN2", target_bir_lowering=False, debug=False,
                   num_devices=N_CORES)

    # xT laid out [q, pp, t, b]: the per-quarter load is a linear 1MB DMA
    xT_ext = nc.dram_tensor("xT", [NQ, P, TB * P], MM_DT,
                            kind="ExternalInput").ap()
    pat_ext = {}
    for k in range(N_FAC):
        for nm in ("patA", "patB"):
            pat_ext[f"{nm}{k}"] = nc.dram_tensor(
                f"{nm}{k}", [P, P], MM_DT, kind="ExternalInput").ap()
    # y staging: [q, s, r*128 + b] bf16; host: y[q*128+b, 32s+r]
    y_ext = nc.dram_tensor("y", [NQ, P, TB * P], MM_DT,
                           kind="ExternalOutput").ap()

    with tile.TileContext(nc) as tc, ExitStack() as ctx:
        const = ctx.enter_context(tc.tile_pool(name="const", bufs=1))
        ps = ctx.enter_context(tc.tile_pool(name="ps", bufs=8, space="PSUM"))
        xtp = ctx.enter_context(tc.tile_pool(name="xtp", bufs=4))
        upool = ctx.enter_context(tc.tile_pool(name="upool", bufs=3))
        vpool = ctx.enter_context(tc.tile_pool(name="vpool", bufs=2))
        ypool = ctx.enter_context(tc.tile_pool(name="ypool", bufs=2))

        patA, patB = [], []
        for k in range(N_FAC):
            pa = const.tile([P, P], MM_DT, tag=f"patA{k}")
            nc.sync.dma_start(pa[:], pat_ext[f"patA{k}"][:])
            pb = const.tile([P, P], MM_DT, tag=f"patB{k}")
            nc.sync.dma_start(pb[:], pat_ext[f"patB{k}"][:])
            patA.append(pa)
            patB.append(pb)

        n_ev = [0]

        def evict(dst, src):
            if n_ev[0] % 2 == 0:
                nc.vector.tensor_copy(dst, src)
            else:
                nc.scalar.copy(dst, src)
            n_ev[0] += 1

        n_tp = [0]

        def dma_transpose(dst, src):
            eng = nc.sync if (tp_mode == "sync" or n_tp[0] % 2 == 0) else nc.scalar
            eng.dma_start_transpose(dst, src)
            n_tp[0] += 1

        def load_x(q):
            # sync ring, same as transposes: SBUF-writing DMAs must never
            # run concurrently with the xbar-transpose S2M path (HW hazard);
            # same-ring FIFO serializes them. All 4 quarters are prefetched
            # up front so transposes have the ring to themselves afterward.
            xT_sb = xtp.tile([P, TB, P], MM_DT, tag="xT", name=f"xT{q}")
            nc.sync.dma_start(xT_sb[:], xT_ext[q])
            return xT_sb

        def stage1(q, xT_sb):
            # one U slab for all 3 factors -> one 3MB transpose per quarter
            U = upool.tile([P, N_FAC * TB * P], MM_DT, tag="U", name=f"U{q}")
            for T in range(TB // 4):
                s1 = [ps.tile([P, 512], F32, tag="ps",
                              name=f"s1_{q}_{T}_{kk}")
                      for kk in range(N_FAC)]
                for tl in range(4):
                    lhsT = xT_sb[:, 4 * T + tl, :]
                    for k in range(N_FAC):
                        nc.tensor.matmul(s1[k][:, ts(tl, P)], lhsT,
                                         patA[k][:], start=True, stop=True)
                # evict: U[b, r*384 + k*128 + t*4 + e] = s1_k[b, tl*128+r*4+e]
                # iterate (r, tl, e): dst runs are 16 contiguous elems (32B)
                for k in range(N_FAC):
                    src = s1[k].rearrange("p (tl r e) -> p r tl e",
                                          tl=4, r=32, e=4)
                    dst = U.rearrange("p (r k t e) -> p r k t e",
                                      k=N_FAC, r=32, t=TB,
                                      e=4)[:, :, k, 4 * T:4 * T + 4]
                    evict(dst, src)
            # corner-turn; r-halves so stage-2 R0-3 can start after the first
            V = vpool.tile([P, TB * N_FAC, P], MM_DT, tag="V", name=f"V{q}")
            half = TB * N_FAC * P // 2
            dma_transpose(V[:, :TB * N_FAC // 2], U[:, :half])
            dma_transpose(V[:, TB * N_FAC // 2:], U[:, half:])
            return V

        def stage2(q, V):
            y_q = ypool.tile([P, TB * P], MM_DT, tag="yq", name=f"yq{q}")
            for R in range(8):
                yps = ps.tile([P, 512], F32, tag="ps", name=f"yps{q}_{R}")
                if s2_mode == "fused":
                    for k in range(N_FAC):
                        for rl in range(4):
                            r = R * 4 + rl
                            nc.tensor.matmul(
                                yps[:, ts(rl, P)],
                                patB[k][:],
                                V[:, r * N_FAC + k, :],
                                start=(k == 0 and rl == 0),
                                stop=(k == N_FAC - 1 and rl == 3),
                                skip_group_check=True)
                else:
                    for rl in range(4):
                        r = R * 4 + rl
                        for k in range(N_FAC):
                            nc.tensor.matmul(
                                yps[:, ts(rl, P)],
                                patB[k][:],
                                V[:, r * N_FAC + k, :],
                                start=(k == 0),
                                stop=(k == N_FAC - 1))
                evict(y_q[:, ts(R, 512)], yps[:])
            nc.scalar.dma_start(y_ext[q], y_q[:])

        # software pipeline with 2-deep stage-1 lookahead so the in-order PE
        # queue always has ready work while corner-turn DMAs drain:
        # s1(0) s1(1) s1(2) s2(0) s1(3) s2(1) s2(2) s2(3)
        xT = [load_x(q) for q in range(NQ)]
        V = [None] * NQ
        V[0] = stage1(0, xT[0])
        V[1] = stage1(1, xT[1])
        V[2] = stage1(2, xT[2])
        stage2(0, V[0])
        V[3] = stage1(3, xT[3])
        stage2(1, V[1])
        stage2(2, V[2])
        stage2(3, V[3])

    nc.compile()
    return nc


_NC_CACHE = {}


def prep_inputs(inputs):
    """Host preprocessing: per-core bf16 quarter-major xT + pattern matrices."""
    import ml_dtypes

    bf16 = ml_dtypes.bfloat16
    x = np.asarray(inputs["input_BI"], dtype=np.float32)
    As = [np.asarray(inputs[n], dtype=np.float32) for n in ("w0a", "w1a", "w2a")]
    Bs = [np.asarray(inputs[n], dtype=np.float32) for n in ("w0b", "w1b", "w2b")]

    common = {}
    # patA_k[pp, r*4+e]; see module docstring for the index map
    pa0 = np.zeros((2, 64, 32, 2, 2), np.float32)      # [g, l, r, g', kob]
    w0 = As[0].reshape(2, 32, 64).transpose(2, 1, 0)   # [l, r, kob]
    for g in range(2):
        pa0[g, :, :, g, :] = w0
    common["patA0"] = pa0.reshape(P, P)
    common["patA1"] = As[1].reshape(4, 32, P).transpose(2, 1, 0).reshape(P, P)
    pa2 = np.zeros((4, 32, 32, 4), np.float32)         # [g, l, r, g']
    for g in range(4):
        pa2[g, :, :, g] = As[2].T                       # [l, r] (ko = r)
    common["patA2"] = pa2.reshape(P, P)

    # patB2_k[p', s]
    pb0 = np.zeros((64, 2, 64, 2), np.float32)         # [j, kob, i, kob']
    for kob in range(2):
        pb0[:, kob, :, kob] = Bs[0].T
    common["patB0"] = pb0.reshape(P, P)
    pb1 = np.zeros((32, 4, 32, 4), np.float32)
    for kob in range(4):
        pb1[:, kob, :, kob] = Bs[1].T
    common["patB1"] = pb1.reshape(P, P)
    common["patB2"] = np.ascontiguousarray(Bs[2].T)

    for k in list(common):
        common[k] = np.ascontiguousarray(common[k].astype(bf16))

    in_maps = []
    for c in range(N_CORES):
        im = dict(common)
        xs = x[c * B_SHARD:(c + 1) * B_SHARD].T.astype(bf16)   # (4096, 512)
        # [q, pp, t, b]: per-quarter DMA is a linear [128, 4096] copy
        im["xT"] = np.ascontiguousarray(
            xs.reshape(TB, P, NQ, P).transpose(2, 1, 0, 3)
            .reshape(NQ, P, TB * P))
        in_maps.append(im)
    return in_maps


def finish_output(res_list, bias):
    """Reassemble [q,s,r,b] bf16 staging -> [B, O] f32 + bias."""
    outs = []
    for r in res_list:
        ystage = np.asarray(r["y"]).reshape(NQ, P, TB, P)
        y_core = ystage.transpose(0, 3, 1, 2).reshape(B_SHARD, O_DIM)
        outs.append(y_core.astype(np.float32))
    y = np.concatenate(outs, axis=0)
    return y + bias[None, :]


def kernel(**inputs):
    """Full-input entry point: shards over B, runs 8-core SPMD, gathers."""
    from concourse.bass_utils import run_bass_kernel_spmd

    in_maps = prep_inputs(inputs)
    if "nc" not in _NC_CACHE:
        _NC_CACHE["nc"] = build_nc()
    res = run_bass_kernel_spmd(_NC_CACHE["nc"], in_maps,
                               core_ids=list(range(N_CORES)))
    bias = np.asarray(inputs["bias_O"], dtype=np.float32)
    return finish_output(res.results, bias)


# revision 29
# speedup vs baseline: 1.4976x; 1.0656x over previous
"""Trainium2 Bass kernel for nn_CustomLayerMKM: y = x @ (sum_k kron(Bk, Ak)).T + bias.

Exploits the Kronecker structure: per factor, y_b = Bk @ X_b @ Ak^T via two
matmul stages with a DMA-xbar corner-turn between them (~9x fewer FLOPs than
dense).

Sharding: data-parallel over B across 8 cores (512 rows each, processed as 4
b-quarters of 128); Kronecker factors replicated. No collectives.

v2 layout (vs v1): stage-2 uses pattern-stationary matmuls whose output
partition is s = o//32 and PSUM tile r = o%32, letting all 3 factors
accumulate into ONE psum bank (single start flag; has_written semantics give
overwrite-then-accumulate). This cuts stage-2 evictions 3x, removes per-u
weight reloads, and the output is written bf16 (y^T staging layout, host
reassembles + bias). Software pipeline interleaves stage-2 of quarter q-1
after stage-1 of quarter q so the PE never waits on the corner-turn DMAs,
which alternate between the two HWDGE queues (sync + scalar).

Index map per factor k with wa:(p,q), wb:(f2,f1), j=i_full//q_k, l=i_full%q_k:
  stage-1 psum col (per i-block t) c = r*4 + e, e encodes (ko//32, j%G)
  U free = r*128 + t*4 + e;  per-128-block DMA transpose -> V[p', r, b]
  p' = t*4+e = {2j+kob | 4j+kob | j} for k={0,1,2}, ko = kob*32 + r
  stage-2: yps[s, b] += patB2_k.T @ V_k[:, r, :], s = o//32, o = 32s + r
"""

from contextlib import ExitStack

import numpy as np

P = 128
B_FULL, I_DIM, O_DIM = 4096, 4096, 4096
N_CORES = 8
B_SHARD = B_FULL // N_CORES          # 512 rows per core
NQ = 4                               # b-shard processed in 4 quarters of 128
N_FAC = 3
TB = I_DIM // P                      # 32 i-blocks
MM_DTYPE = "bfloat16"


def build_nc(tp_mode="sync", s2_mode="fused"):
    """tp_mode: 'split' (half-tiles, alternate sync/scalar HWDGE),
    'alt' (full tiles, alternate engines), 'sync' (full tiles, sync only).
    s2_mode: 'fused' (one psum group for all 3 factors, single start) or
    'groups' (per-rl accumulation groups, baseline-proven semantics)."""
    import concourse.bass as bass
    import concourse.mybir as mybir
    import concourse.tile as tile
    from concourse import bacc

    MM_DT = getattr(mybir.dt, MM_DTYPE)
    F32 = mybir.dt.float32
    ts = bass.ts

    nc = bacc.Bacc("TRN2", target_bir_lowering=False, debug=False,
                   num_devices=N_CORES)

    # xT laid out [q, pp, t, b]: the per-quarter load is a linear 1MB DMA
    xT_ext = nc.dram_tensor("xT", [NQ, P, TB * P], MM_DT,
                            kind="ExternalInput").ap()
    pat_ext = {}
    for k in range(N_FAC):
        for nm in ("patA", "patB"):
            pat_ext[f"{nm}{k}"] = nc.dram_tensor(
                f"{nm}{k}", [P, P], MM_DT, kind="ExternalInput").ap()
    # y staging: [q, s, r*128 + b] bf16; host: y[q*128+b, 32s+r]
    y_ext = nc.dram_tensor("y", [NQ, P, TB * P], MM_DT,
                           kind="ExternalOutput").ap()

    with tile.TileContext(nc) as tc, ExitStack() as ctx:
        const = ctx.enter_context(tc.tile_pool(name="const", bufs=1))
        ps1 = ctx.enter_context(tc.tile_pool(name="ps1", bufs=6, space="PSUM"))
        ps2 = ctx.enter_context(tc.tile_pool(name="ps2", bufs=2, space="PSUM"))
        xtp = ctx.enter_context(tc.tile_pool(name="xtp", bufs=4))
        upool = ctx.enter_context(tc.tile_pool(name="upool", bufs=3))
        vpool = ctx.enter_context(tc.tile_pool(name="vpool", bufs=2))
        ypool = ctx.enter_context(tc.tile_pool(name="ypool", bufs=2))

        def load_patterns():
            patA, patB = [], []
            for k in range(N_FAC):
                pa = const.tile([P, P], MM_DT, tag=f"patA{k}")
                nc.sync.dma_start(pa[:], pat_ext[f"patA{k}"][:])
                pb = const.tile([P, P], MM_DT, tag=f"patB{k}")
                nc.sync.dma_start(pb[:], pat_ext[f"patB{k}"][:])
                patA.append(pa)
                patB.append(pb)
            return patA, patB

        n_ev = [0]

        def evict(dst, src):
            if n_ev[0] % 2 == 0:
                nc.vector.tensor_copy(dst, src)
            else:
                nc.scalar.copy(dst, src)
            n_ev[0] += 1

        n_tp = [0]

        def dma_transpose(dst, src):
            eng = nc.sync if (tp_mode == "sync" or n_tp[0] % 2 == 0) else nc.scalar
            eng.dma_start_transpose(dst, src)
            n_tp[0] += 1

        def load_x(q, pieces=1):
            # sync ring, same as transposes: SBUF-writing DMAs must never
            # run concurrently with the xbar-transpose S2M path (HW hazard);
            # same-ring FIFO serializes them. All 4 quarters are prefetched
            # up front so transposes have the ring to themselves afterward.
            xT_sb = xtp.tile([P, TB, P], MM_DT, tag="xT", name=f"xT{q}")
            w = TB // pieces
            for pc in range(pieces):
                nc.sync.dma_start(xT_sb[:, pc * w:(pc + 1) * w],
                                  xT_ext[q][:, pc * w * P:(pc + 1) * w * P])
            return xT_sb

        def stage1(q, xT_sb, tp_pieces=2):
            # one U slab for all 3 factors -> one transpose pair per quarter
            U = upool.tile([P, N_FAC * TB * P], MM_DT, tag="U", name=f"U{q}")
            for T in range(TB // 4):
                s1 = [ps1.tile([P, 512], F32, tag="ps",
                               name=f"s1_{q}_{T}_{kk}")
                      for kk in range(N_FAC)]
                for tl in range(4):
                    lhsT = xT_sb[:, 4 * T + tl, :]
                    for k in range(N_FAC):
                        nc.tensor.matmul(s1[k][:, ts(tl, P)], lhsT,
                                         patA[k][:], start=True, stop=True)
                # evict: U[b, r*384 + k*128 + t*4 + e] = s1_k[b, tl*128+r*4+e]
                # iterate (r, tl, e): dst runs are 16 contiguous elems (32B)
                for k in range(N_FAC):
                    src = s1[k].rearrange("p (tl r e) -> p r tl e",
                                          tl=4, r=32, e=4)
                    dst = U.rearrange("p (r k t e) -> p r k t e",
                                      k=N_FAC, r=32, t=TB,
                                      e=4)[:, :, k, 4 * T:4 * T + 4]
                    evict(dst, src)
            # corner-turn; r-pieces so stage-2 R-quads start before the rest
            V = vpool.tile([P, TB * N_FAC, P], MM_DT, tag="V", name=f"V{q}")
            w = TB * N_FAC // tp_pieces
            for pc in range(tp_pieces):
                dma_transpose(V[:, pc * w:(pc + 1) * w],
                              U[:, pc * w * P:(pc + 1) * w * P])
            return V

        def stage2(q, V):
            y_q = ypool.tile([P, TB * P], MM_DT, tag="yq", name=f"yq{q}")
            for R in range(8):
                yps = ps2.tile([P, 512], F32, tag="ps2", name=f"yps{q}_{R}")
                if s2_mode == "fused":
                    for k in range(N_FAC):
                        for rl in range(4):
                            r = R * 4 + rl
                            nc.tensor.matmul(
                                yps[:, ts(rl, P)],
                                patB[k][:],
                                V[:, r * N_FAC + k, :],
                                start=(k == 0 and rl == 0),
                                stop=(k == N_FAC - 1 and rl == 3),
                                skip_group_check=True)
                else:
                    for rl in range(4):
                        r = R * 4 + rl
                        for k in range(N_FAC):
                            nc.tensor.matmul(
                                yps[:, ts(rl, P)],
                                patB[k][:],
                                V[:, r * N_FAC + k, :],
                                start=(k == 0),
                                stop=(k == N_FAC - 1))
                evict(y_q[:, ts(R, 512)], yps[:])
            nc.scalar.dma_start(y_ext[q], y_q[:])

        # Software pipeline, 2-deep stage-1 lookahead; separate psum pools
        # (ps1/ps2) keep stage-2 psum tiles off stage-1's recycle ring so the
        # in-order engine queues never head-of-line block on late V tiles.
        xT = [None] * NQ
        xT[0] = load_x(0, pieces=4)
        patA, patB = load_patterns()
        for q in range(1, NQ):
            xT[q] = load_x(q)
        V = [None] * NQ
        V[0] = stage1(0, xT[0])
        V[1] = stage1(1, xT[1])
        V[2] = stage1(2, xT[2])
        stage2(0, V[0])
        V[3] = stage1(3, xT[3], tp_pieces=4)
        stage2(1, V[1])
        stage2(2, V[2])
        stage2(3, V[3])

    nc.compile()
    return nc


_NC_CACHE = {}


def prep_inputs(inputs):
    """Host preprocessing: per-core bf16 quarter-major xT + pattern matrices."""
    import ml_dtypes

    bf16 = ml_dtypes.bfloat16
    x = np.asarray(inputs["input_BI"], dtype=np.float32)
    As = [np.asarray(inputs[n], dtype=np.float32) for n in ("w0a", "w1a", "w2a")]
    Bs = [np.asarray(inputs[n], dtype=np.float32) for n in ("w0b", "w1b", "w2b")]

    common = {}
    # patA_k[pp, r*4+e]; see module docstring for the index map
    pa0 = np.zeros((2, 64, 32, 2, 2), np.float32)      # [g, l, r, g', kob]
    w0 = As[0].reshape(2, 32, 64).transpose(2, 1, 0)   # [l, r, kob]
    for g in range(2):
        pa0[g, :, :, g, :] = w0
    common["patA0"] = pa0.reshape(P, P)
    common["patA1"] = As[1].reshape(4, 32, P).transpose(2, 1, 0).reshape(P, P)
    pa2 = np.zeros((4, 32, 32, 4), np.float32)         # [g, l, r, g']
    for g in range(4):
        pa2[g, :, :, g] = As[2].T                       # [l, r] (ko = r)
    common["patA2"] = pa2.reshape(P, P)

    # patB2_k[p', s]
    pb0 = np.zeros((64, 2, 64, 2), np.float32)         # [j, kob, i, kob']
    for kob in range(2):
        pb0[:, kob, :, kob] = Bs[0].T
    common["patB0"] = pb0.reshape(P, P)
    pb1 = np.zeros((32, 4, 32, 4), np.float32)
    for kob in range(4):
        pb1[:, kob, :, kob] = Bs[1].T
    common["patB1"] = pb1.reshape(P, P)
    common["patB2"] = np.ascontiguousarray(Bs[2].T)

    for k in list(common):
        common[k] = np.ascontiguousarray(common[k].astype(bf16))

    in_maps = []
    for c in range(N_CORES):
        im = dict(common)
        xs = x[c * B_SHARD:(c + 1) * B_SHARD].T.astype(bf16)   # (4096, 512)
        # [q, pp, t, b]: per-quarter DMA is a linear [128, 4096] copy
        im["xT"] = np.ascontiguousarray(
            xs.reshape(TB, P, NQ, P).transpose(2, 1, 0, 3)
            .reshape(NQ, P, TB * P))
        in_maps.append(im)
    return in_maps


def finish_output(res_list, bias):
    """Reassemble [q,s,r,b] bf16 staging -> [B, O] f32 + bias."""
    outs = []
    for r in res_list:
        ystage = np.asarray(r["y"]).reshape(NQ, P, TB, P)
        y_core = ystage.transpose(0, 3, 1, 2).reshape(B_SHARD, O_DIM)
        outs.append(y_core.astype(np.float32))
    y = np.concatenate(outs, axis=0)
    return y + bias[None, :]


def kernel(**inputs):
    """Full-input entry point: shards over B, runs 8-core SPMD, gathers."""
    from concourse.bass_utils import run_bass_kernel_spmd

    in_maps = prep_inputs(inputs)
    if "nc" not in _NC_CACHE:
        _NC_CACHE["nc"] = build_nc()
    res = run_bass_kernel_spmd(_NC_CACHE["nc"], in_maps,
                               core_ids=list(range(N_CORES)))
    bias = np.asarray(inputs["bias_O"], dtype=np.float32)
    return finish_output(res.results, bias)
